# revision 49
# baseline (speedup 1.0000x reference)
"""Gated multi-head attention on 8 NeuronCores — v2.

Sharding (hardcoded): core c -> (batch b = c // 4, head-group g = c % 4).
Data-parallel over B=2, tensor-parallel over the 16 heads in groups of 4.
Each core computes its 4 heads' attention plus the corresponding slice of
the output projection; the host sums the 4 head-group partials per batch
and adds the output bias.

Per-core kernel v2 (all matmuls bf16):
  qT[256,2048], kT[256,2048] projections (DVE eviction with scale/bias)
  v[2048, 4, 65] natural [pos, head, dim] layout, ones column at dim 64
  per head-pair c, 512-query block:
    S^T[k,q] = kT.T-chunks @ qT   row-tiled 2-way (K=64 heads on PE row
               groups 0 / 64, concurrent)
    P^T = exp(S^T)  (ACT, [128,1024] per instr; no max-subtraction)
    per head: A^T_raw[65,512] = sum_kc (V_h|1)^T-stationary @ P^T-chunks
              (row 64 = softmax denominator)
  per 512-query block: recip of 4 denominator rows in one DVE op,
    DMA-broadcast across 64 partitions, DVE-normalize A^T to bf16
  y[q,1024] = A^T-stationary @ Wo  (no transposes needed)
"""

import math
from contextlib import ExitStack

import numpy as np

import concourse.bass as bass
import concourse.tile as tile
from concourse import mybir
from concourse.bass_utils import run_bass_kernel_spmd

B = 2
N = 2048
E = 1024
H = 16
D = 64
NCORES = 8
GROUPS = NCORES // B      # head-groups per batch
HG = H // GROUPS          # heads per core
DH = HG * D               # 256 head-dims per core
P = 128

F32 = mybir.dt.float32
BF16 = mybir.dt.bfloat16
AF = mybir.ActivationFunctionType

TRACE = False
LAST_RESULTS = None


def _split_drain_waits(nc):
    """The installed walrus build accepts only ONE sync-wait per instruction
    (one NEURON_ISA_TPB_EVENTS slot), but Tile emits several on drains,
    matmuls, etc.  Hoist all but the last wait onto dedicated single-wait
    NOPs ahead of the instruction on the same engine (the lowering newer
    walrus performs itself)."""
    n = 0
    for fn in nc.m.functions:
        for bb in fn.blocks:
            insts = bb.instructions
            idx = 0
            while idx < len(insts):
                inst = insts[idx]
                si = inst.sync_info
                if si is not None and len(si.on_wait) > 1:
                    waits = list(si.on_wait)
                    nops = []
                    for w in waits[:-1]:
                        n += 1
                        nop = mybir.InstNoOp(
                            name=f"waitsplit-{n}",
                            engine=inst.engine,
                            sync_info=mybir.SyncInfo(on_wait=[w], on_update=[]),
                            bass_nofuse=True,
                        )
                        nc.register_instruction(nop)
                        nops.append(nop)
                    inst.sync_info = mybir.SyncInfo(
                        on_wait=[waits[-1]], on_update=list(si.on_update))
                    insts[idx:idx] = nops
                    idx += len(nops)
                idx += 1
    return n


def _build():
    nc = bass.Bass()
    xqT = nc.dram_tensor("xqT", [E, N], BF16, kind="ExternalInput")
    xkT = nc.dram_tensor("xkT", [E, N], BF16, kind="ExternalInput")
    xvT = nc.dram_tensor("xvT", [E, N], BF16, kind="ExternalInput")
    wqT = nc.dram_tensor("wqT", [E, DH], BF16, kind="ExternalInput")
    wkT = nc.dram_tensor("wkT", [E, DH], BF16, kind="ExternalInput")
    wvT = nc.dram_tensor("wvT", [E, DH], BF16, kind="ExternalInput")
    woB = nc.dram_tensor("woB", [DH, E], BF16, kind="ExternalInput")
    qscale = nc.dram_tensor("qscale", [DH], F32, kind="ExternalInput")
    qbias = nc.dram_tensor("qbias", [DH], F32, kind="ExternalInput")
    kbias = nc.dram_tensor("kbias", [DH], F32, kind="ExternalInput")
    vbias = nc.dram_tensor("vbias", [DH], F32, kind="ExternalInput")
    y = nc.dram_tensor("y", [N, E], F32, kind="ExternalOutput")
    dscr = nc.dram_tensor("dscr", [16, 512], F32, kind="Internal")
    dscr2 = nc.dram_tensor("dscr2", [16, 512], F32, kind="Internal")

    KC = E // P            # 8 contraction chunks over the embed dim
    MC = DH // P           # 2 head-pair chunks over this core's head dims
    NB = N // 512          # 4 query blocks
    KB = N // P            # 16 key-position chunks

    with ExitStack() as ctx:
        tc = ctx.enter_context(tile.TileContext(nc))
        const = ctx.enter_context(tc.tile_pool(name="const", bufs=1))
        xpool = ctx.enter_context(tc.tile_pool(name="xpool", bufs=1))
        xqpool = ctx.enter_context(tc.tile_pool(name="xqpool", bufs=1))
        wpool = ctx.enter_context(tc.tile_pool(name="wpool", bufs=3))
        wopool = ctx.enter_context(tc.tile_pool(name="wopool", bufs=MC))
        qkpool = ctx.enter_context(tc.tile_pool(name="qkpool", bufs=2 * MC))
        vpool = ctx.enter_context(tc.tile_pool(name="vpool", bufs=KB))
        ptpool = ctx.enter_context(tc.tile_pool(name="ptpool", bufs=19))
        arpool = ctx.enter_context(tc.tile_pool(name="arpool", bufs=3))
        atpool = ctx.enter_context(tc.tile_pool(name="atpool", bufs=4))
        dnpool = ctx.enter_context(tc.tile_pool(name="dnpool", bufs=2))
        rcpool = ctx.enter_context(tc.tile_pool(name="rcpool", bufs=3))
        bcpool = ctx.enter_context(tc.tile_pool(name="bcpool", bufs=3))
        ypool = ctx.enter_context(tc.tile_pool(name="ypool", bufs=2))
        pp = ctx.enter_context(tc.tile_pool(name="pp", bufs=2, space="PSUM"))
        stq = ctx.enter_context(tc.tile_pool(name="stq", bufs=2, space="PSUM"))
        pss = ctx.enter_context(tc.tile_pool(name="pss", bufs=2, space="PSUM"))

        def load_w(w_dram, tag, eng):
            # Whole weight tensor in one DMA: [E, DH] -> [128, KC, DH]
            t = wpool.tile([P, KC, DH], BF16, name=tag)
            ap = w_dram[:, :]
            src = bass.AP(tensor=ap.tensor, offset=0,
                          ap=[[DH, P], [P * DH, KC], [1, DH]])
            eng.dma_start(out=t, in_=src)
            return [t[:, kc, :] for kc in range(KC)]

        def load_x(x_dram, pool, eng):
            # Whole activation tensor in one DMA: [E, N] -> [128, KC, N]
            t = pool.tile([P, KC, N], BF16, name="xall")
            ap = x_dram[:, :]
            src = bass.AP(tensor=ap.tensor, offset=0,
                          ap=[[N, P], [P * N, KC], [1, N]])
            eng.dma_start(out=t, in_=src)
            return [t[:, kc, :] for kc in range(KC)]

        # --- transposed projections: out[c][dd, n], DVE eviction ---
        def emit_proj_block(o, xs, w_c, c, nb, scale_sb, bias_sb):
            pt = pp.tile([P, 512], F32, name="pp")
            for kc in range(KC):
                nc.tensor.matmul(
                    pt,
                    lhsT=w_c[kc][:, c * P:(c + 1) * P],
                    rhs=xs[kc][:, nb * 512:(nb + 1) * 512],
                    start=(kc == 0), stop=(kc == KC - 1))
            if scale_sb is not None:
                nc.vector.tensor_scalar(
                    out=o[:, nb * 512:(nb + 1) * 512], in0=pt,
                    scalar1=scale_sb[:, c:c + 1],
                    scalar2=bias_sb[:, c:c + 1],
                    op0=mybir.AluOpType.mult,
                    op1=mybir.AluOpType.add)
            else:
                nc.vector.tensor_scalar_add(
                    out=o[:, nb * 512:(nb + 1) * 512], in0=pt,
                    scalar1=bias_sb[:, c:c + 1])

        # HAM warm-up: junk matmuls (zeroed SBUF -> scratch PSUM, never
        # read) keep the PE busy through the initial input-DMA wait, so the
        # clock gate is at K=8/8 when the real projections start.
        junk = const.tile([P, P], BF16, name="warm")
        nc.gpsimd.memset(junk, 0.0)
        wps = pp.tile([P, 512], F32, name="pp")
        for _ in range(300):
            nc.tensor.matmul(wps[:, 0:P], lhsT=junk, rhs=junk,
                             start=True, stop=True)

        wk_c = load_w(wkT, "wk", nc.gpsimd)
        xk = load_x(xkT, xpool, nc.sync)

        qs_sb = const.tile([P, MC], F32, name="qs")
        nc.sync.dma_start(out=qs_sb, in_=qscale[:].rearrange("(c p) -> p c", p=P))
        qb_sb = const.tile([P, MC], F32, name="qb")
        nc.sync.dma_start(out=qb_sb, in_=qbias[:].rearrange("(c p) -> p c", p=P))
        kb_sb = const.tile([P, MC], F32, name="kb")
        nc.sync.dma_start(out=kb_sb, in_=kbias[:].rearrange("(c p) -> p c", p=P))
        vb_ap = vbias[:]
        vb_bc = const.tile([P, DH], F32, name="vb")
        nc.gpsimd.dma_start(out=vb_bc, in_=bass.AP(
            tensor=vb_ap.tensor, offset=vb_ap.offset, ap=[[0, P]] + vb_ap.ap))

        wq_c = load_w(wqT, "wq", nc.gpsimd)
        xq = load_x(xqT, xqpool, nc.gpsimd)
        wv_c = load_w(wvT, "wv", nc.gpsimd)
        xv = load_x(xvT, xpool, nc.sync)
        wo_sb = []
        for c in range(MC):
            t = wopool.tile([P, E], BF16, name="wo")
            nc.sync.dma_start(out=t, in_=woB[c * P:(c + 1) * P, :])
            wo_sb.append(t)

        kT = [qkpool.tile([P, N], BF16, name="kt") for _ in range(MC)]
        qT = [qkpool.tile([P, N], BF16, name="qt") for _ in range(MC)]
        # Inline: only the first blocks unit (0,0) needs immediately; the
        # remaining kT[0] blocks are emitted just-in-time between its S
        # chunks, and everything else goes through the fill queue.
        emit_proj_block(kT[0], xk, wk_c, 0, 0, None, kb_sb)
        emit_proj_block(qT[0], xq, wq_c, 0, 0, qs_sb, qb_sb)

        # --- v in natural [pos, head, dim+1] layout, bf16, ones col ---
        v_sb = {}

        def emit_vchunk(m):
            vt = vpool.tile([P, HG, D + 1], BF16, name="vt")
            nc.gpsimd.memset(vt[:, :, D:D + 1], 1.0)
            pv = pp.tile([P, 512], F32, name="pp")[:, :DH]
            for kc in range(KC):
                nc.tensor.matmul(
                    pv,
                    lhsT=xv[kc][:, m * P:(m + 1) * P],
                    rhs=wv_c[kc],
                    start=(kc == 0), stop=(kc == KC - 1))
            nc.vector.tensor_add(
                out=vt[:, :, 0:D],
                in0=pv.rearrange("p (h d) -> p h d", h=HG),
                in1=vb_bc.rearrange("p (h d) -> p h d", h=HG))
            v_sb[m] = vt

        # --- attention pipeline ---
        # units: (pair c, 512-query block qb), pair-fast so that both pairs
        # of a query block finish adjacently (recip + O-proj batch per qb).
        units = [(c, qb) for qb in range(NB) for c in range(MC)]
        pts = {}            # (unit, kc) -> P^T tile [128, 1024]
        avps = {}           # unit -> (avp_h0, avp_h1) PSUM [65, 512]
        araw = {}           # unit -> raw A^T tile [128, 512] bf16 (2 heads)
        dns = {}            # qb -> denominator tile [4, 512] f32
        ats = {}            # unit -> normalized A^T tile [128, 512] bf16

        def emit_s_chunk(u, kc):
            c, qb = u
            ps = stq.tile([P, 1024], F32, name="stq")
            q0 = qb * 512
            k0 = kc * P
            nc.tensor.matmul(
                ps[:, 0:512],
                lhsT=kT[c][0:64, k0:k0 + P],
                rhs=qT[c][0:64, q0:q0 + 512],
                start=True, stop=True, tile_position=(0, 0))
            nc.tensor.matmul(
                ps[:, 512:1024],
                lhsT=kT[c][64:128, k0:k0 + P],
                rhs=qT[c][64:128, q0:q0 + 512],
                start=True, stop=True, tile_position=(64, 0))
            ptile = ptpool.tile([P, 1024], BF16, name="pt")
            nc.scalar.activation(out=ptile, in_=ps, func=AF.Exp)
            pts[(u, kc)] = ptile

        def emit_av_chunk(u, kc):
            c, qb = u
            if u not in avps:
                avps[u] = (pss.tile([P, 512], F32, name="pss"),
                           pss.tile([P, 512], F32, name="pss"))
            pt = pts[(u, kc)]
            for hh in range(2):
                h = 2 * c + hh
                nc.tensor.matmul(
                    avps[u][hh][0:D + 1, :],
                    lhsT=v_sb[kc][:, h, :],
                    rhs=pt[:, hh * 512:(hh + 1) * 512],
                    start=(kc == 0), stop=(kc == KB - 1))
            del pts[(u, kc)]

        dn2s = {}

        def emit_av_done(u):
            # Evict raw A^T (bf16) and the f32 denominator rows (scattered
            # into the q-on-partitions dn2 tile), freeing the two PSUM
            # accumulators.
            c, qb = u
            ar = arpool.tile([P, 512], BF16, name="ar")
            if qb not in dn2s:
                dn2s[qb] = dnpool.tile([P, 4, 4], F32, name="dn2")
            for hh in range(2):
                nc.vector.tensor_copy(
                    out=ar[hh * D:(hh + 1) * D, :], in_=avps[u][hh][0:D, :])
                rr = rcpool.tile([1, 512], F32, name="rr")
                nc.vector.tensor_copy(out=rr, in_=avps[u][hh][D:D + 1, :])
                row = qb * 4 + 2 * c + hh
                nc.sync.dma_start(out=dn2s[qb][:, 2 * c + hh, :], in_=rr)
                dns[(u, hh)] = row
            araw[u] = ar
            del avps[u]

        def emit_norm(qb):
            # Reciprocal of all 4 heads' denominators of this query block in
            # a q-on-partitions [128, 4, 4] layout (cheap: 16 elems/lane),
            # then broadcast each head's row across 64 partitions and
            # normalize the raw A^T into its O-projection layout.
            dn2 = dn2s.pop(qb)
            rc2 = rcpool.tile([P, 4, 4], F32, name="rc2")
            nc.vector.reciprocal(out=rc2, in_=dn2)
            dap = dscr2[qb * 4:qb * 4 + 4, :]
            dst = bass.AP(tensor=dap.tensor, offset=dap.offset,
                          ap=[[4, P], [512, 4], [1, 4]])
            nc.sync.dma_start(out=dst, in_=rc2)
            for c in range(MC):
                at = atpool.tile([P, 512], BF16, name="at")
                bc = bcpool.tile([P, 512], F32, name="bc")
                ats[(c, qb)] = at
                for hh in range(2):
                    row = dns.pop(((c, qb), hh))
                    r = dscr2[row:row + 1, :]
                    nc.gpsimd.dma_start(
                        out=bc[hh * D:(hh + 1) * D, :],
                        in_=bass.AP(
                            tensor=r.tensor, offset=r.offset,
                            ap=[[0, D]] + r.ap[1:]))
                    nc.vector.tensor_mul(
                        out=at[hh * D:(hh + 1) * D, :],
                        in0=araw[(c, qb)][hh * D:(hh + 1) * D, :],
                        in1=bc[hh * D:(hh + 1) * D, :])
                del araw[(c, qb)]

        def emit_oproj(qb, ql):
            # y rows [qb*512 + ql*128, +128) = A[q,:] @ woB
            q0 = qb * 512 + ql * P
            for nn in range(2):
                py = pp.tile([P, 512], F32, name="pp")
                for c in range(MC):
                    nc.tensor.matmul(
                        py,
                        lhsT=ats[(c, qb)][:, ql * P:(ql + 1) * P],
                        rhs=wo_sb[c][:, nn * 512:(nn + 1) * 512],
                        start=(c == 0), stop=(c == MC - 1))
                yt = ypool.tile([P, 512], F32, name="yt")
                nc.vector.tensor_copy(out=yt, in_=py)
                nc.sync.dma_start(
                    out=y[q0:q0 + P, nn * 512:(nn + 1) * 512], in_=yt)
            if ql == 3:
                del ats[(0, qb)], ats[(1, qb)]

        from collections import deque
        fill = deque()
        for nb in range(1, NB):
            fill.append(lambda nb=nb: emit_proj_block(
                qT[0], xq, wq_c, 0, nb, qs_sb, qb_sb))
        for nb in range(NB):
            fill.append(lambda nb=nb: emit_proj_block(
                kT[1], xk, wk_c, 1, nb, None, kb_sb))
        for nb in range(NB):
            fill.append(lambda nb=nb: emit_proj_block(
                qT[1], xq, wq_c, 1, nb, qs_sb, qb_sb))
        for m in range(KB):
            fill.append(lambda m=m: emit_vchunk(m))

        next_nb = 1
        for i, u in enumerate(units):
            last = i == len(units) - 1
            av_ptr = 0
            for kc in range(KB):
                if i == 0 and kc < 3:
                    emit_proj_block(kT[0], xk, wk_c, 0, kc + 1, None, kb_sb)
                emit_s_chunk(u, kc)
                if last and kc == 9:
                    while fill:
                        fill.popleft()()
                if last and kc >= 10:
                    for _ in range(3):
                        if av_ptr < kc:
                            emit_av_chunk(u, av_ptr)
                            av_ptr += 1
                    continue
                if fill:
                    fill.popleft()()
                if len(fill) > 16 and fill:
                    fill.popleft()()
                if len(fill) > 28 and fill:
                    fill.popleft()()
            if last:
                while av_ptr < KB:
                    emit_av_chunk(u, av_ptr)
                    av_ptr += 1
                emit_av_done(u)
            else:
                for kc in range(KB):
                    fill.append(lambda u=u, kc=kc: emit_av_chunk(u, kc))
                fill.append(lambda u=u: emit_av_done(u))
            if u[0] == MC - 1:
                qb = u[1]
                fill.append(lambda qb=qb: emit_norm(qb))
                for ql in range(4):
                    fill.append(lambda qb=qb, ql=ql: emit_oproj(qb, ql))
        while fill:
            fill.popleft()()

    _split_drain_waits(nc)
    return nc


_CACHE = {}


def _get_nc():
    if "nc" not in _CACHE:
        _CACHE["nc"] = _build()
    return _CACHE["nc"]


def kernel(query, key, value, Wq, bq, Wk, bk, Wv, bv, Wo, bo, gate):
    global LAST_RESULTS
    query = np.asarray(query, np.float32)
    key = np.asarray(key, np.float32)
    value = np.asarray(value, np.float32)
    Wq = np.asarray(Wq, np.float32)
    Wk = np.asarray(Wk, np.float32)
    Wv = np.asarray(Wv, np.float32)
    Wo = np.asarray(Wo, np.float32)
    bq = np.asarray(bq, np.float32)
    bk = np.asarray(bk, np.float32)
    bv = np.asarray(bv, np.float32)
    bo = np.asarray(bo, np.float32)
    gate = np.asarray(gate, np.float32)

    scale_h = (1.0 / (1.0 + np.exp(-gate.astype(np.float64)))
               / math.sqrt(D)).astype(np.float32)

    xq_b = [np.ascontiguousarray(query[b].T) for b in range(B)]
    xk_b = [np.ascontiguousarray(key[b].T) for b in range(B)]
    xv_b = [np.ascontiguousarray(value[b].T) for b in range(B)]

    in_maps = []
    for core in range(NCORES):
        b, g = divmod(core, GROUPS)
        rows = slice(g * DH, (g + 1) * DH)
        qs = np.repeat(scale_h[g * HG:(g + 1) * HG], D)
        in_maps.append({
            "xqT": xq_b[b], "xkT": xk_b[b], "xvT": xv_b[b],
            "wqT": np.ascontiguousarray(Wq[rows].T),
            "wkT": np.ascontiguousarray(Wk[rows].T),
            "wvT": np.ascontiguousarray(Wv[rows].T),
            "woB": np.ascontiguousarray(Wo[:, rows].T),
            "qscale": np.ascontiguousarray(qs),
            "qbias": np.ascontiguousarray(bq[rows] * qs),
            "kbias": np.ascontiguousarray(bk[rows]),
            "vbias": np.ascontiguousarray(bv[rows]),
        })

    from concourse import mybir as _mb
    bf = _mb.dt.np(_mb.dt.bfloat16)
    for m in in_maps:
        for k in ("xqT", "xkT", "xvT", "wqT", "wkT", "wvT", "woB"):
            m[k] = m[k].astype(bf)
    res = run_bass_kernel_spmd(_get_nc(), in_maps, list(range(NCORES)),
                               trace=TRACE)
    LAST_RESULTS = res
    out = np.empty((B, N, E), np.float32)
    for b in range(B):
        acc = res.results[b * GROUPS]["y"].astype(np.float32).copy()
        for g in range(1, GROUPS):
            acc += res.results[b * GROUPS + g]["y"]
        out[b] = acc + bo
    return out
